# revision 25
# baseline (speedup 1.0000x reference)
"""Multi-head attention (B=2, S=2048, D=1024, H=16) on 8 TRN2 NeuronCores.

Sharding: 2D tensor-parallel — batch (2-way) x head-groups (4-way).
Core c handles batch c//4 and heads [4*(c%4), 4*(c%4)+4).

Per-core device program (bf16 streaming operands, f32 PSUM accumulation):
  1. QT/KT = (W^T x X^T) in feature-major layout [dk=256, S], bf16
  2. V in token-major [S, dv=256], stored bf16 per-head with a ones column
     (the ones column yields the softmax denominator Z via the P@V matmul)
  3. scores^T[k, q] = K^T Q, both heads of a pair in one [128,1024] PSUM
     tile via zero-padded per-head Q (full-K matmuls keep the PE array
     fully active and share one LDWEIGHTS); one exp per k-tile on ACT
     (scale=1/8 folded in); multiplicative bf16 keep-mask on partially-
     masked tiles only; fully-masked tiles skipped (host analyzes the
     mask, so causal masks get block sparsity for free)
  4. xu^T[d, q] (+ Z row) = V'^T P^T accumulated over k-tiles; 1/Z computed
     on a [128, 4] redistribution of the Z row (DVE is partition-parallel),
     broadcast across partitions on GpSimd
  5. out[tok, of] = x Wo + bo (bias fed as zeros on non-group-0 cores);
     host sums the 4 per-group partials of each batch.
"""

import numpy as np
import ml_dtypes

B, S, D, H = 2, 2048, 1024, 16
DK = 64
NCORES = 8
GROUPS = 4  # head groups (cores per batch)
HPC = H // GROUPS  # heads per core = 4
DH = HPC * DK  # head dims per core = 256

KT_TILES = S // 128  # 16 k tiles of 128
QC_CHUNKS = S // 512  # 4 q chunks of 512
TOKC = S // 512  # 4 token chunks of 512
DC = D // 128  # 8 contraction chunks for projections


_prog_cache: dict = {}


def _build_program(active, partial):
    """active: tuple(len QC_CHUNKS) of tuples of kt indices; partial: frozenset[(kt, qc)]."""
    import concourse.bass as bass
    import concourse.tile as tile
    from concourse import bacc, mybir

    dt = mybir.dt
    f32, f32r, bf16 = dt.float32, dt.float32r, dt.bfloat16
    AF = mybir.ActivationFunctionType
    ALU = mybir.AluOpType

    nc = bacc.Bacc("TRN2", target_bir_lowering=False, debug=False, num_devices=NCORES)

    xtq = nc.dram_tensor("xtq", [D, S], bf16, kind="ExternalInput").ap()
    xtk = nc.dram_tensor("xtk", [D, S], bf16, kind="ExternalInput").ap()
    xtv = nc.dram_tensor("xtv", [D, S], bf16, kind="ExternalInput").ap()
    wq_d = nc.dram_tensor("wq", [128, DC, DH], bf16, kind="ExternalInput").ap()
    wk_d = nc.dram_tensor("wk", [128, DC, DH], bf16, kind="ExternalInput").ap()
    wv_d = nc.dram_tensor("wv", [128, DC, DH], bf16, kind="ExternalInput").ap()
    wo_d = nc.dram_tensor("wo", [128, 2, D], bf16, kind="ExternalInput").ap()
    bq_d = nc.dram_tensor("bq", [128, 2], f32, kind="ExternalInput").ap()
    bk_d = nc.dram_tensor("bk", [128, 2], f32, kind="ExternalInput").ap()
    bv_d = nc.dram_tensor("bv", [128, HPC, DK], f32, kind="ExternalInput").ap()
    bo_d = nc.dram_tensor("bo", [128, D], f32, kind="ExternalInput").ap()
    keep_d = nc.dram_tensor("keep", [S, S], bf16, kind="ExternalInput").ap()
    ones_d = nc.dram_tensor("ones", [1, 128], f32r, kind="ExternalInput").ap()
    out_d = nc.dram_tensor("out", [S, D], f32, kind="ExternalOutput").ap()

    with tile.TileContext(nc) as tc:
        with (
            tc.tile_pool(name="persist", bufs=1) as persist,
            tc.tile_pool(name="pt_pool", bufs=18) as pt_pool,
            tc.tile_pool(name="kp_pool", bufs=4) as kp_pool,
            tc.tile_pool(name="norm", bufs=3) as norm,
            tc.tile_pool(name="out_pool", bufs=2) as out_pool,
            tc.tile_pool(name="psB", bufs=3, space=bass.MemorySpace.PSUM) as psB,
            tc.tile_pool(name="psPV", bufs=2, space=bass.MemorySpace.PSUM) as psPV,
        ):
            # --- persistent tiles (fine-grained so consumers start early) ---
            QTz = [
                [persist.tile([128, 512], bf16, tag=f"qtz{h}_{t}", name=f"qtz{h}_{t}") for t in range(TOKC)]
                for h in range(HPC)
            ]
            KT = [
                [persist.tile([128, 512], bf16, tag=f"kt{i}_{t}", name=f"kt{i}_{t}") for t in range(TOKC)]
                for i in range(2)
            ]
            xT = [
                [persist.tile([128, 512], bf16, tag=f"xt{i}_{t}", name=f"xt{i}_{t}") for t in range(TOKC)]
                for i in range(2)
            ]
            # flat per-k-tile V': head h at cols [h*65, h*65+65) (64 V dims + ones);
            # tail padded so a 128-wide lhsT slice from any head stays in bounds
            vp = [persist.tile([128, 336], bf16, tag=f"vp{t}", name=f"vp{t}") for t in range(KT_TILES)]
            bo_sb = persist.tile([128, D], f32, tag="bo")
            bv_sb = persist.tile([128, HPC, DK], f32, tag="bv")
            bq_sb = persist.tile([128, 2], f32, tag="bq")
            bk_sb = persist.tile([128, 2], f32, tag="bk")
            wo_sb = persist.tile([128, 2, D], bf16, tag="wo")

            nc.gpsimd.dma_start(bo_sb[:], bo_d[:])
            nc.gpsimd.dma_start(bv_sb[:], bv_d[:])
            nc.gpsimd.dma_start(bq_sb[:], bq_d[:])
            nc.gpsimd.dma_start(bk_sb[:], bk_d[:])
            nc.gpsimd.dma_start(wo_sb[:], wo_d[:])
            for h in range(HPC):
                op = 64 if (h % 2) == 0 else 0
                for t in range(TOKC):
                    nc.vector.memset(QTz[h][t][op : op + 64, :], 0.0)
            for t in range(KT_TILES):
                nc.vector.memset(vp[t][:, 260:336], 0.0)

            # --- stage 1: projections, K/V/Q interleaved per half-sequence ---
            with (
                tc.tile_pool(name="wpool", bufs=1) as wpool,
                tc.tile_pool(name="xblk", bufs=24) as xblk,
            ):
                w_sb = {}
                for name, wd in (("k", wk_d), ("v", wv_d), ("q", wq_d)):
                    w_sb[name] = wpool.tile([128, DC, DH], bf16, tag=f"w{name}", name=f"w{name}")
                    nc.sync.dma_start(w_sb[name][:], wd[:])

                def k_proj(tokc, xb):
                    tks = slice((tokc % 2) * 512, (tokc % 2) * 512 + 512)
                    for dkt in range(2):
                        ps = psB.tile([128, 512], f32, tag="big", name="kproj_ps")
                        for dc in range(DC):
                            nc.tensor.matmul(
                                ps[:],
                                w_sb["k"][:, dc, dkt * 128 : dkt * 128 + 128],
                                xb[dc][:, tks],
                                start=(dc == 0),
                                stop=(dc == DC - 1),
                            )
                        nc.vector.tensor_scalar(
                            KT[dkt][tokc][:], ps[:], bk_sb[:, dkt : dkt + 1], None, ALU.add
                        )

                def q_proj(tokc, xb):
                    tks = slice((tokc % 2) * 512, (tokc % 2) * 512 + 512)
                    for dkt in range(2):
                        ps = psB.tile([128, 512], f32, tag="big", name="qproj_ps")
                        for dc in range(DC):
                            nc.tensor.matmul(
                                ps[:],
                                w_sb["q"][:, dc, dkt * 128 : dkt * 128 + 128],
                                xb[dc][:, tks],
                                start=(dc == 0),
                                stop=(dc == DC - 1),
                            )
                        for sub in range(2):
                            po = sub * 64
                            nc.vector.tensor_scalar(
                                QTz[2 * dkt + sub][tokc][po : po + 64, :],
                                ps[po : po + 64, :],
                                bq_sb[po : po + 64, dkt : dkt + 1],
                                None,
                                ALU.add,
                            )

                def v_proj(tokc, xb):
                    for tt in range(4):
                        tokt = tokc * 4 + tt
                        lo = (tokc % 2) * 512 + tt * 128
                        ps = psB.tile([128, HPC, DK], f32, tag="big", name="vproj_ps")
                        for dc in range(DC):
                            nc.tensor.matmul(
                                ps[:],
                                xb[dc][:, lo : lo + 128],
                                w_sb["v"][:, dc, :],
                                start=(dc == 0),
                                stop=(dc == DC - 1),
                            )
                        vp3 = vp[tokt][:, 0 : HPC * 65].rearrange("p (h d) -> p h d", d=DK + 1)
                        nc.vector.tensor_add(vp3[:, :, 0:DK], ps[:], bv_sb[:])
                        nc.vector.memset(vp3[:, :, DK : DK + 1], 1.0)

                for half in range(2):
                    cs = slice(half * 1024, half * 1024 + 1024)
                    blocks = {}
                    for name, xt_d in (("k", xtk), ("v", xtv), ("q", xtq)):
                        blocks[name] = []
                        for dc in range(DC):
                            t = xblk.tile([128, 1024], bf16, tag="xb", name="xb")
                            nc.sync.dma_start(t[:], xt_d[dc * 128 : dc * 128 + 128, cs])
                            blocks[name].append(t)
                    for tokc in (2 * half, 2 * half + 1):
                        k_proj(tokc, blocks["k"])
                        v_proj(tokc, blocks["v"])
                        q_proj(tokc, blocks["q"])

            # --- stage 2: attention (ht outer); paired heads share one [128,1024] score psum ---
            def normalize(pvs, ht, po, qc):
                """xT[head dims, qc] = pvs[0:64] * (1/Z); Z = pvs row 64."""
                # Z row [1, 512] -> [128, 4] so reciprocal uses all DVE lanes
                zrow = norm.tile([1, 512], f32, tag="zrow", name="zrow")
                nc.vector.tensor_copy(zrow[:], pvs[DK : DK + 1, :])
                zc = norm.tile([128, 4], f32, tag="zc", name="zc")
                nc.gpsimd.dma_start(zc[:], zrow[:])
                rzc = norm.tile([128, 4], f32, tag="rzc", name="rzc")
                nc.vector.reciprocal(rzc[:], zc[:])
                rz = norm.tile([1, 512], f32, tag="rz", name="rz")
                nc.gpsimd.dma_start(rz[:], rzc[:])
                rzb = norm.tile([64, 512], f32, tag="rzb_sb", name="rzb_sb")
                nc.gpsimd.partition_broadcast(rzb[:], rz[:])
                nc.vector.tensor_mul(xT[ht][qc][po : po + 64, :], pvs[0:DK, :], rzb[:])

            for ht in range(2):
                for qc in range(QC_CHUNKS):
                    kts = active[qc]
                    if not kts:
                        continue
                    qs = slice(qc * 512, qc * 512 + 512)
                    pts = {}
                    for kt in kts:
                        sps = psB.tile([128, 1024], f32, tag="big", name="score_ps")
                        for sub in range(2):  # head pair shares the KT lhsT
                            nc.tensor.matmul(
                                sps[:, sub * 512 : sub * 512 + 512],
                                KT[ht][kt // 4][:, (kt % 4) * 128 : (kt % 4) * 128 + 128],
                                QTz[2 * ht + sub][qc][:],
                                start=True,
                                stop=True,
                            )
                        pt = pt_pool.tile([128, 1024], bf16, tag="pt", name="pt")
                        nc.scalar.activation(pt[:], sps[:], AF.Exp, scale=0.125)
                        if (kt, qc) in partial:
                            kp = kp_pool.tile([128, 512], bf16, tag="kp", name="kp")
                            nc.gpsimd.dma_start(
                                kp[:], keep_d[kt * 128 : kt * 128 + 128, qs]
                            )
                            for sub in range(2):
                                hs2 = slice(sub * 512, sub * 512 + 512)
                                nc.vector.tensor_mul(pt[:, hs2], pt[:, hs2], kp[:])
                        pts[kt] = pt
                    for sub in range(2):
                        h = ht * 2 + sub
                        po = sub * 64
                        pvs = psPV.tile([128, 512], f32, tag="pv", name="pv_ps")
                        for i, kt in enumerate(kts):
                            nc.tensor.matmul(
                                pvs[:],
                                vp[kt][:, h * 65 : h * 65 + 128],
                                pts[kt][:, sub * 512 : sub * 512 + 512],
                                start=(i == 0),
                                stop=(i == len(kts) - 1),
                            )
                        normalize(pvs, ht, po, qc)

            # --- stage 3: output projection [tok, of] ---
            for tokt in range(S // 128):
                osb = out_pool.tile([128, D], f32, tag="osb", name="osb")
                for ofc in range(2):
                    ops = psB.tile([128, 512], f32, tag="big", name="oproj_ps")
                    for half in range(2):
                        nc.tensor.matmul(
                            ops[:],
                            xT[half][tokt // 4][:, (tokt % 4) * 128 : (tokt % 4) * 128 + 128],
                            wo_sb[:, half, ofc * 512 : ofc * 512 + 512],
                            start=(half == 0),
                            stop=(half == 1),
                        )
                    nc.vector.tensor_add(
                        osb[:, ofc * 512 : ofc * 512 + 512],
                        ops[:],
                        bo_sb[:, ofc * 512 : ofc * 512 + 512],
                    )
                eng = nc.sync if tokt % 2 == 0 else nc.gpsimd
                eng.dma_start(out_d[tokt * 128 : tokt * 128 + 128, :], osb[:])

    nc.compile()
    return nc


def _mask_structure(mask):
    """Per-(kt, qc) tile status of keep^T = (~mask)^T."""
    keep_t = (~np.asarray(mask)).T  # [k, q]
    view = keep_t.reshape(KT_TILES, 128, QC_CHUNKS, 512)
    t_any = view.any(axis=(1, 3))
    t_all = view.all(axis=(1, 3))
    active = tuple(
        tuple(kt for kt in range(KT_TILES) if t_any[kt, qc]) for qc in range(QC_CHUNKS)
    )
    partial = frozenset(
        (kt, qc)
        for qc in range(QC_CHUNKS)
        for kt in active[qc]
        if not t_all[kt, qc]
    )
    return keep_t, active, partial


def _bf16(a):
    return np.ascontiguousarray(np.asarray(a, np.float32)).astype(ml_dtypes.bfloat16)


def _prep_inputs(query, key_in, value, mask, Wq, bq, Wk, bk, Wv, bv, Wo, bo):
    keep_t, active, partial = _mask_structure(mask)
    keep_bf16 = np.ascontiguousarray(keep_t).astype(ml_dtypes.bfloat16)

    def wshape(w, sl):  # [D, DH] -> [128, DC, DH]
        return _bf16(np.asarray(w[:, sl]).reshape(DC, 128, DH).transpose(1, 0, 2))

    in_maps = []
    for c in range(NCORES):
        b, g = c // GROUPS, c % GROUPS
        hs = slice(g * DH, (g + 1) * DH)
        m = {
            "xtq": _bf16(np.asarray(query[b]).T),
            "xtk": _bf16(np.asarray(key_in[b]).T),
            "xtv": _bf16(np.asarray(value[b]).T),
            "wq": wshape(Wq, hs),
            "wk": wshape(Wk, hs),
            "wv": wshape(Wv, hs),
            "wo": _bf16(np.asarray(Wo[hs, :]).reshape(2, 128, D).transpose(1, 0, 2)),
            "bq": np.ascontiguousarray(np.asarray(bq[hs]).reshape(2, 128).T),
            "bk": np.ascontiguousarray(np.asarray(bk[hs]).reshape(2, 128).T),
            "bv": np.broadcast_to(
                np.asarray(bv[hs]).reshape(HPC, DK), (128, HPC, DK)
            ).copy(),
            "bo": (
                np.broadcast_to(np.asarray(bo), (128, D)).copy()
                if g == 0
                else np.zeros((128, D), np.float32)
            ),
            "keep": keep_bf16,
            "ones": np.ones((1, 128), np.float32),
        }
        in_maps.append({k: np.ascontiguousarray(v) for k, v in m.items()})
    return in_maps, active, partial


def kernel(query, key_in, value, mask, Wq, bq, Wk, bk, Wv, bv, Wo, bo, _trace=False):
    from concourse.bass_utils import run_bass_kernel_spmd

    in_maps, active, partial = _prep_inputs(
        query, key_in, value, mask, Wq, bq, Wk, bk, Wv, bv, Wo, bo
    )
    key = (active, partial)
    if key not in _prog_cache:
        _prog_cache[key] = _build_program(active, partial)
    nc = _prog_cache[key]

    res = run_bass_kernel_spmd(nc, in_maps, list(range(NCORES)), trace=_trace)
    kernel.last_result = res

    out = np.zeros((B, S, D), np.float32)
    for c in range(NCORES):
        out[c // GROUPS] += res.results[c]["out"]
    return out
